# revision 10
# baseline (speedup 1.0000x reference)
"""Trainium2 Bass kernel for MaskedMicrolensingTransformer.

Sharding: 8 cores = 4 batch elements x 2 token-halves (750 tokens each,
padded to 768).  Per layer each core LayerNorms its own tokens, the LN
output (transposed, bf16) is AllGather'd within its pair, and each core
computes K/V for all 1536 tokens but Q/attention/FFN only for its own
768 tokens.  Masked-sum pooling partials are returned per core and the
tiny [4,256] head epilogue runs on the host.

Attention: cosine attention with |scores| <= 1/sqrt(32) << 5, so the
reference clip is a no-op.  Softmax runs without max-subtraction:
u = exp(SCALE*cos + ln(mask)) straight from the scores PSUM (key-side
mask folded into the per-partition exp bias), denominators via
ones-block col-tiled matmuls accumulated alongside u@v, and
o = (u @ v) * exp(-ln(denom)).
"""

import os
import numpy as np

import concourse.bacc as bacc
import concourse.tile as tile
import concourse.mybir as mybir
from concourse.bass_utils import run_bass_kernel_spmd

F32 = mybir.dt.float32
BF16 = mybir.dt.bfloat16
ALU = mybir.AluOpType
ACT = mybir.ActivationFunctionType

B, T, D, H, FF, L = 4, 1500, 256, 8, 1024, 4
DK = D // H
SCALE = 1.0 / float(np.sqrt(DK))
TL = T // 2            # 750 real tokens per core
TP = 768               # padded tokens per core
S = 2 * TP             # padded full sequence (1536)
NT = TP // 128         # 6 token tiles
ND = D // 128          # 2 d tiles
NS = S // 128          # 12 s tiles
NFF = FF // 128        # 8 ff tiles
NCORES = 8
EPS = 1e-5
CH768 = ((0, 512), (512, 256))
CH1536 = ((0, 512), (512, 512), (1024, 512))

DEBUG = bool(int(os.environ.get("KERNEL_DEBUG", "0")))

_CACHE = {}


def _ln_np(x, g, b, eps=EPS):
    m = x.mean(-1, keepdims=True)
    v = ((x - m) ** 2).mean(-1, keepdims=True)
    return (x - m) / np.sqrt(v + eps) * g + b


def _bf(a):
    import ml_dtypes
    return np.ascontiguousarray(np.asarray(a, np.float32)).astype(ml_dtypes.bfloat16)


def _build_program():
    nc = bacc.Bacc("TRN2", target_bir_lowering=False, debug=False,
                   num_devices=NCORES)

    def din(name, shape, dt=F32):
        return nc.dram_tensor(name, list(shape), dt, kind="ExternalInput")

    # ---- per-core inputs
    xcol = din("xcol", [128, NT])
    vmrow = din("vmrow", [1, TP], BF16)
    vmsb = din("vmsb", [128, NT], BF16)
    vmsbf = din("vmsbf", [128, NT])
    lnm = din("lnm", [128, NS])
    pos_s = din("pos_s", [TP, D], BF16)

    # ---- shared weights / consts
    w_in_rep = din("w_in_rep", [128, D])
    inb_rep = din("inb_rep", [128, D])
    inlng_rep = din("inlng_rep", [128, D])
    inlnb_rep = din("inlnb_rep", [128, D])
    ve_w1col = din("ve_w1col", [64, 1])
    ve_b1col = din("ve_b1col", [64, 1])
    ve_w2 = din("ve_w2", [64, D], BF16)
    ve_b2row = din("ve_b2row", [1, D], BF16)
    ng_rep = din("ng_rep", [128, D])
    nb_rep = din("nb_rep", [128, D])
    mhrep = din("mhrep", [128, 128], BF16)
    onesb32 = din("onesb32", [128, 32], BF16)
    ones1_64 = din("ones1_64", [1, 64], BF16)
    ones1_128 = din("ones1_128", [1, 128], BF16)

    lwd = []
    for l in range(L):
        lwd.append(dict(
            qw=din(f"qw{l}", [ND, 128, D], BF16),
            kw=din(f"kw{l}", [ND, 128, D], BF16),
            vw=din(f"vw{l}", [ND, 128, D], BF16),
            ow=din(f"ow{l}", [ND, 128, D], BF16),
            owlo=din(f"owlo{l}", [ND, 128, D], BF16),
            f1w=din(f"f1w{l}", [ND, 128, FF], BF16),
            f2w=din(f"f2w{l}", [NFF, 128, D], BF16),
            f2wlo=din(f"f2wlo{l}", [NFF, 128, D], BF16),
            qb2=din(f"qb2_{l}", [128, ND]),
            kb2=din(f"kb2_{l}", [128, ND]),
            obp=din(f"obp{l}", [1, D], BF16),
            f1b8=din(f"f1b8_{l}", [128, NFF]),
            f2brow=din(f"f2brow{l}", [1, D], BF16),
            g1rep=din(f"g1rep{l}", [128, D]),
            b1rep=din(f"b1rep{l}", [128, D]),
            g2rep=din(f"g2rep{l}", [128, D]),
            b2rep=din(f"b2rep{l}", [128, D]),
        ))

    pooled_out = nc.dram_tensor("pooled", [1, D], F32, kind="ExternalOutput")
    taps = {}
    if DEBUG:
        def tap(name, shape):
            taps[name] = nc.dram_tensor(name, list(shape), F32, kind="ExternalOutput")
        tap("h_emb", [TP, D])
        for l in range(L):
            tap(f"h_l{l}", [TP, D])
        def tapb(name, shape):
            taps[name] = nc.dram_tensor(name, list(shape), BF16, kind="ExternalOutput")
        tapb("qh0", [128, ND * TP])
        tapb("kh0", [128, ND * S])
        tapb("v0", [128, NS * D])
        tapb("ot0", [128, 2 * TP])
        tap("den0", [128, 2 * TP])

    agin = [nc.dram_tensor(f"agin{l}", [128, ND * TP], BF16) for l in range(L)]
    agout = [nc.dram_tensor(f"agout{l}", [2, 128, ND * TP], BF16) for l in range(L)]
    RG = [[0, 1], [2, 3], [4, 5], [6, 7]]

    with tile.TileContext(nc) as tc:
        with (
            tc.tile_pool(name="wts", bufs=1) as wts,
            tc.tile_pool(name="act", bufs=1) as acts,
            tc.tile_pool(name="tmp", bufs=2) as tmp,
            tc.tile_pool(name="tmp1", bufs=1) as tmp1,
            tc.tile_pool(name="u", bufs=3) as upool,
            tc.tile_pool(name="pbig", bufs=2, space="PSUM") as pbig,   # [128,1024] x2 = 4 banks
            tc.tile_pool(name="pav", bufs=2, space="PSUM") as pav,     # [128,512] x2 = 2 banks
            tc.tile_pool(name="psm", bufs=2, space="PSUM") as psm,     # [128,256] x2 = 2 banks
        ):
            # ---------- persistent loads ----------
            def load2(dram, shape, dt=F32):
                t = wts.tile(list(shape), dt, tag=dram.name)
                nc.sync.dma_start(t[:], dram.ap())
                return t

            def load3(dram, shape, dt=BF16):
                # dram [n, 128, w] -> sbuf [128, n, w]
                t = wts.tile([128, shape[0], shape[2]], dt, tag=dram.name)
                for i in range(shape[0]):
                    nc.sync.dma_start(t[:, i, :], dram.ap()[i])
                return t

            xcol_t = load2(xcol, [128, NT])
            vmrow_t = load2(vmrow, [1, TP], BF16)
            vmsb_t = load2(vmsb, [128, NT], BF16)
            vmsbf_t = load2(vmsbf, [128, NT])
            lnm_t = load2(lnm, [128, NS])
            w_in_t = load2(w_in_rep, [128, D])
            inb_t = load2(inb_rep, [128, D])
            inlng_t = load2(inlng_rep, [128, D])
            inlnb_t = load2(inlnb_rep, [128, D])
            vw1_t = load2(ve_w1col, [64, 1])
            vb1_t = load2(ve_b1col, [64, 1])
            vw2_t = load2(ve_w2, [64, D], BF16)
            vb2_t = load2(ve_b2row, [1, D], BF16)
            ng_t = load2(ng_rep, [128, D])
            nb_t = load2(nb_rep, [128, D])
            mhrep_t = load2(mhrep, [128, 128], BF16)
            onesb32_t = load2(onesb32, [128, 32], BF16)
            ones64_t = load2(ones1_64, [1, 64], BF16)
            ones128_t = load2(ones1_128, [1, 128], BF16)
            pos_t = wts.tile([128, NT, D], BF16, tag="pos_t")
            for tt in range(NT):
                nc.sync.dma_start(pos_t[:, tt, :],
                                  pos_s.ap()[128 * tt:128 * (tt + 1), :])
            LWT = []
            for l in range(L):
                w = lwd[l]
                LWT.append(dict(
                    qw=load3(w["qw"], [ND, 128, D]),
                    kw=load3(w["kw"], [ND, 128, D]),
                    vw=load3(w["vw"], [ND, 128, D]),
                    ow=load3(w["ow"], [ND, 128, D]),
                    owlo=load3(w["owlo"], [ND, 128, D]),
                    f1w=load3(w["f1w"], [ND, 128, FF]),
                    f2w=load3(w["f2w"], [NFF, 128, D]),
                    f2wlo=load3(w["f2wlo"], [NFF, 128, D]),
                    qb2=load2(w["qb2"], [128, ND]),
                    kb2=load2(w["kb2"], [128, ND]),
                    obp=load2(w["obp"], [1, D], BF16),
                    f1b8=load2(w["f1b8"], [128, NFF]),
                    f2brow=load2(w["f2brow"], [1, D], BF16),
                    g1rep=load2(w["g1rep"], [128, D]),
                    b1rep=load2(w["b1rep"], [128, D]),
                    g2rep=load2(w["g2rep"], [128, D]),
                    b2rep=load2(w["b2rep"], [128, D]),
                ))

            epscol = wts.tile([128, 1], F32, tag="epscol")
            nc.vector.memset(epscol[:], EPS)
            tinycol = wts.tile([128, 1], F32, tag="tinycol")
            nc.vector.memset(tinycol[:], 1e-16)

            # ---------- helpers ----------
            def ln_block(h_big, g_t, b_t, out_dt, tag, pool=None):
                """h_big: [128, NT, D] tile -> returns [128, NT, D] tile."""
                bags = tmp.tile([128, NT, 2], F32, tag=f"bags_{tag}")
                for tt in range(NT):
                    bst = tmp.tile([128, 6], F32, tag=f"bst_{tag}")
                    nc.vector.bn_stats(bst[:], h_big[:, tt, :])
                    nc.vector.bn_aggr(bags[:, tt, :], bst[:])
                lnv = tmp.tile([128, NT], F32, tag=f"lnv_{tag}")
                nc.scalar.activation(lnv[:], bags[:, :, 1], ACT.Ln,
                                     bias=epscol[:], scale=1.0)
                rstds = tmp.tile([128, NT], F32, tag=f"rstd_{tag}")
                nc.scalar.activation(rstds[:], lnv[:], ACT.Exp,
                                     bias=0.0, scale=-0.5)
                out = (pool or tmp1).tile([128, NT, D], out_dt, tag=f"lnout_{tag}")
                for tt in range(NT):
                    t1 = tmp.tile([128, D], F32, tag=f"lnt1_{tag}")
                    nc.vector.scalar_tensor_tensor(
                        t1[:], h_big[:, tt, :], bags[:, tt, 0:1], g_t[:],
                        op0=ALU.subtract, op1=ALU.mult)
                    nc.vector.scalar_tensor_tensor(
                        out[:, tt, :], t1[:], rstds[:, tt:tt + 1], b_t[:],
                        op0=ALU.mult, op1=ALU.add)
                return out

            # ---------- embedding ----------
            vmrep = pbig.tile([128, 1024], F32, tag="pbig")
            for off, w in CH768:
                nc.tensor.matmul(vmrep[0:64, off:off + w], ones64_t[:],
                                 vmrow_t[:, off:off + w], start=True, stop=True)
            tmpT = acts.tile([64, TP], BF16, tag="vf_tmpT")
            nc.scalar.activation(tmpT[:], vmrep[0:64, 0:TP], ACT.Gelu,
                                 bias=vb1_t[:], scale=vw1_t[:])
            emb_big = acts.tile([128, NT, D], F32, tag="hB")
            for tt in range(NT):
                nc.vector.scalar_tensor_tensor(
                    emb_big[:, tt, :], w_in_t[:], xcol_t[:, tt:tt + 1], inb_t[:],
                    op0=ALU.mult, op1=ALU.add)
            xe_big = ln_block(emb_big, inlng_t, inlnb_t, F32, "lnf")
            h_big = acts.tile([128, NT, D], F32, tag="hA")
            for tt in range(NT):
                pvf = psm.tile([128, D], F32, tag="psm")
                nc.tensor.matmul(pvf[:], tmpT[:, 128 * tt:128 * (tt + 1)],
                                 vw2_t[:], start=True, stop=False)
                nc.tensor.matmul(pvf[:], ones128_t[:], vb2_t[:],
                                 start=False, stop=True)
                t1 = tmp.tile([128, D], F32, tag="embt1")
                nc.vector.tensor_add(t1[:], pvf[:], xe_big[:, tt, :])
                nc.vector.tensor_add(h_big[:, tt, :], t1[:], pos_t[:, tt, :])

            if DEBUG:
                for tt in range(NT):
                    nc.sync.dma_start(taps["h_emb"].ap()[128 * tt:128 * (tt + 1), :],
                                      h_big[:, tt, :])

            # ---------- layers ----------
            for l in range(L):
                W = LWT[l]
                a_big = ln_block(h_big, W["g1rep"], W["b1rep"], BF16, "ln1")
                aTown = acts.tile([128, ND, TP], BF16, tag="aTown")
                for tt in range(NT):
                    for dc in range(ND):
                        nc.sync.dma_start_transpose(
                            aTown[:, dc, 128 * tt:128 * (tt + 1)],
                            a_big[:, tt, 128 * dc:128 * (dc + 1)])
                nc.sync.dma_start(agin[l].ap(),
                                  aTown[:].rearrange("a b c -> a (b c)"))
                nc.gpsimd.collective_compute(
                    "AllGather", ALU.bypass, replica_groups=RG,
                    ins=[agin[l].ap()], outs=[agout[l].ap()])
                aTfull = acts.tile([128, ND, S], BF16, tag="aTfull")
                for r in range(2):
                    for dc in range(ND):
                        nc.sync.dma_start(
                            aTfull[:, dc, TP * r:TP * (r + 1)],
                            agout[l].ap()[r, :, TP * dc:TP * (dc + 1)])

                # ---- q/k projection + cosine normalization (chunk-wise)
                def qk_proj(wt, bcol2, src, width, chunks, outname):
                    hat = acts.tile([128, ND, width], BF16, tag=outname)
                    for et in range(ND):
                        for off, cw in chunks:
                            pq = pav.tile([128, 512], F32, tag="pav")
                            for dt_ in range(ND):
                                nc.tensor.matmul(
                                    pq[:, 0:cw],
                                    wt[:, dt_, 128 * et:128 * (et + 1)],
                                    src[:, dt_, off:off + cw],
                                    start=(dt_ == 0), stop=(dt_ == ND - 1))
                            sq = tmp.tile([128, 512], BF16, tag="sq_qk")
                            nc.scalar.activation(sq[:, 0:cw], pq[:, 0:cw],
                                                 ACT.Square,
                                                 bias=bcol2[:, et:et + 1], scale=1.0)
                            pn = pav.tile([128, 512], F32, tag="pav")
                            nc.tensor.matmul(pn[:, 0:cw], mhrep_t[:], sq[:, 0:cw],
                                             start=True, stop=True)
                            lnn = tmp.tile([128, 512], F32, tag="lnrc_qk")
                            nc.scalar.activation(lnn[:, 0:cw], pn[:, 0:cw], ACT.Ln,
                                                 bias=tinycol[:], scale=1.0)
                            rn = tmp.tile([128, 512], F32, tag="lnrc_qk")
                            nc.scalar.activation(rn[:, 0:cw], lnn[:, 0:cw], ACT.Exp,
                                                 bias=0.0, scale=-0.5)
                            nc.vector.scalar_tensor_tensor(
                                hat[:, et, off:off + cw], pq[:, 0:cw],
                                bcol2[:, et:et + 1], rn[:, 0:cw],
                                op0=ALU.add, op1=ALU.mult)
                    return hat

                qhat = qk_proj(W["qw"], W["qb2"], aTown, TP, CH768, "qhat")
                khat = qk_proj(W["kw"], W["kb2"], aTfull, S, CH1536, "khat")

                # ---- v projection
                vt = acts.tile([128, NS, D], BF16, tag="vt")
                for st in range(NS):
                    pv = psm.tile([128, D], F32, tag="psm")
                    for dt_ in range(ND):
                        nc.tensor.matmul(
                            pv[:], aTfull[:, dt_, 128 * st:128 * (st + 1)],
                            W["vw"][:, dt_, :],
                            start=(dt_ == 0), stop=(dt_ == ND - 1))
                    nc.vector.tensor_copy(vt[:, st, :], pv[:])

                if DEBUG and l == 0:
                    nc.sync.dma_start(taps["qh0"].ap(), qhat[:].rearrange("a b c -> a (b c)"))
                    nc.sync.dma_start(taps["kh0"].ap(), khat[:].rearrange("a b c -> a (b c)"))
                    nc.sync.dma_start(taps["v0"].ap(), vt[:].rearrange("a b c -> a (b c)"))

                # ---- attention
                oTn = acts.tile([128, 2, TP], BF16, tag="oTn")
                for grp in range(2):
                    for toff, tw in CH768:
                        avp = pav.tile([128, 512], F32, tag="pav")
                        dnp = pav.tile([128, 512], F32, tag="pav")
                        for st in range(NS):
                            spA = pbig.tile([128, 1024], F32, tag="pbig")
                            spB = pbig.tile([128, 1024], F32, tag="pbig")
                            for hh in range(4):
                                h8 = 4 * grp + hh
                                sp = spA if hh < 2 else spB
                                coff = 512 * (hh % 2)
                                nc.tensor.matmul(
                                    sp[:, coff:coff + tw],
                                    khat[32 * hh:32 * (hh + 1), grp, 128 * st:128 * (st + 1)],
                                    qhat[32 * hh:32 * (hh + 1), grp, toff:toff + tw],
                                    start=True, stop=True,
                                    tile_position=(32 * hh, 0))
                            uA = upool.tile([128, 2, 512], BF16, tag="u")
                            uB = upool.tile([128, 2, 512], BF16, tag="u")
                            nc.scalar.activation(
                                uA[:, :, 0:tw],
                                spA[:].rearrange("a (h b) -> a h b", h=2)[:, :, 0:tw],
                                ACT.Exp, bias=lnm_t[:, st:st + 1], scale=SCALE)
                            nc.scalar.activation(
                                uB[:, :, 0:tw],
                                spB[:].rearrange("a (h b) -> a h b", h=2)[:, :, 0:tw],
                                ACT.Exp, bias=lnm_t[:, st:st + 1], scale=SCALE)
                            for j in range(4):
                                uu = uA if j < 2 else uB
                                us = uu[:, j % 2, 0:tw]
                                nc.tensor.matmul(
                                    avp[32 * j:32 * (j + 1), 0:tw],
                                    vt[:, st, 32 * (4 * grp + j):32 * (4 * grp + j + 1)],
                                    us, start=(st == 0), stop=(st == NS - 1),
                                    tile_position=(0, 32 * j), skip_group_check=True)
                                nc.tensor.matmul(
                                    dnp[32 * j:32 * (j + 1), 0:tw],
                                    onesb32_t[:], us,
                                    start=(st == 0), stop=(st == NS - 1),
                                    tile_position=(0, 32 * j), skip_group_check=True)
                        lnd = tmp.tile([128, 512], F32, tag="lnrc_at")
                        nc.scalar.activation(lnd[:, 0:tw], dnp[:, 0:tw], ACT.Ln,
                                             bias=0.0, scale=1.0)
                        rcd = tmp.tile([128, 512], F32, tag="lnrc_at")
                        nc.scalar.activation(rcd[:, 0:tw], lnd[:, 0:tw], ACT.Exp,
                                             bias=0.0, scale=-1.0)
                        nc.vector.tensor_mul(oTn[:, grp, toff:toff + tw],
                                             avp[:, 0:tw], rcd[:, 0:tw])
                        if DEBUG and l == 0:
                            dd = tmp1.tile([128, 512], F32, tag="dbgden")
                            nc.vector.tensor_copy(dd[:, 0:tw], dnp[:, 0:tw])
                            nc.sync.dma_start(
                                taps["den0"].ap()[:, TP * grp + toff:TP * grp + toff + tw],
                                dd[:, 0:tw])

                if DEBUG and l == 0:
                    nc.sync.dma_start(taps["ot0"].ap(), oTn[:].rearrange("a b c -> a (b c)"))

                # ---- O projection + residual (gate1*0.5 folded into ow/obp)
                h_mid = acts.tile([128, NT, D], F32, tag="hB")
                for tt in range(NT):
                    po = psm.tile([128, D], F32, tag="psm")
                    for grp in range(ND):
                        nc.tensor.matmul(po[:], oTn[:, grp, 128 * tt:128 * (tt + 1)],
                                         W["ow"][:, grp, :],
                                         start=(grp == 0), stop=False)
                        nc.tensor.matmul(po[:], oTn[:, grp, 128 * tt:128 * (tt + 1)],
                                         W["owlo"][:, grp, :],
                                         start=False, stop=False)
                    nc.tensor.matmul(po[:], ones128_t[:], W["obp"][:],
                                     start=False, stop=True)
                    nc.vector.tensor_add(h_mid[:, tt, :], po[:], h_big[:, tt, :])

                # ---- FFN (gate2 folded into f2w/f2b)
                a2_big = ln_block(h_mid, W["g2rep"], W["b2rep"], BF16, "ln2")
                a2T = acts.tile([128, ND, TP], BF16, tag="aTown")
                for tt in range(NT):
                    for dc in range(ND):
                        nc.sync.dma_start_transpose(
                            a2T[:, dc, 128 * tt:128 * (tt + 1)],
                            a2_big[:, tt, 128 * dc:128 * (dc + 1)])
                fT = acts.tile([128, NFF, TP], BF16, tag="fT")
                for ft in range(NFF):
                    pf = pbig.tile([128, 1024], F32, tag="pbig")
                    for off, wd in CH768:
                        for dt_ in range(ND):
                            nc.tensor.matmul(
                                pf[:, off:off + wd],
                                W["f1w"][:, dt_, 128 * ft:128 * (ft + 1)],
                                a2T[:, dt_, off:off + wd],
                                start=(dt_ == 0), stop=(dt_ == ND - 1))
                    nc.scalar.activation(fT[:, ft, :], pf[:, 0:TP], ACT.Gelu,
                                         bias=W["f1b8"][:, ft:ft + 1], scale=1.0)
                h_next = acts.tile([128, NT, D], F32, tag="hA")
                for tt in range(NT):
                    p2 = psm.tile([128, D], F32, tag="psm")
                    for ft in range(NFF):
                        nc.tensor.matmul(p2[:], fT[:, ft, 128 * tt:128 * (tt + 1)],
                                         W["f2w"][:, ft, :],
                                         start=(ft == 0), stop=False)
                        nc.tensor.matmul(p2[:], fT[:, ft, 128 * tt:128 * (tt + 1)],
                                         W["f2wlo"][:, ft, :],
                                         start=False, stop=False)
                    nc.tensor.matmul(p2[:], ones128_t[:], W["f2brow"][:],
                                     start=False, stop=True)
                    t1 = tmp.tile([128, D], F32, tag="ffnt1")
                    nc.vector.tensor_add(t1[:], p2[:], h_mid[:, tt, :])
                    nc.vector.tensor_scalar(h_next[:, tt, :], t1[:], 10.0, -10.0,
                                            op0=ALU.min, op1=ALU.max)
                h_big = h_next

                if DEBUG:
                    for tt in range(NT):
                        nc.sync.dma_start(
                            taps[f"h_l{l}"].ap()[128 * tt:128 * (tt + 1), :],
                            h_big[:, tt, :])

            # ---------- final norm + masked-sum pooling ----------
            hN = ln_block(h_big, ng_t, nb_t, F32, "lnf")
            pp = psm.tile([128, D], F32, tag="psm")
            for tt in range(NT):
                nc.tensor.matmul(pp[0:1, :], vmsbf_t[:, tt:tt + 1], hN[:, tt, :],
                                 start=(tt == 0), stop=(tt == NT - 1))
            pooled_sb = tmp.tile([1, D], F32, tag="pooled")
            nc.vector.tensor_copy(pooled_sb[:], pp[0:1, :])
            nc.sync.dma_start(pooled_out.ap(), pooled_sb[:])

    nc.finalize()
    return nc


def _prep_inputs(x, validity_mask, params):
    p = {k: np.asarray(v, np.float32) for k, v in params.items()}
    x = np.asarray(x, np.float32)
    vm = np.asarray(validity_mask).astype(np.float32)

    def rep(v):
        return np.broadcast_to(np.asarray(v, np.float32)[None, :], (128, D)).copy()

    shared = {
        "w_in_rep": np.broadcast_to(p["in_w"][0][None, :], (128, D)).copy(),
        "inb_rep": rep(p["in_b"]),
        "inlng_rep": rep(p["in_ln_g"]),
        "inlnb_rep": rep(p["in_ln_b"]),
        "ve_w1col": np.ascontiguousarray(p["ve_w1"][0][:, None]),
        "ve_b1col": np.ascontiguousarray(p["ve_b1"][:, None]),
        "ve_w2": _bf(p["ve_w2"]),
        "ve_b2row": _bf(p["ve_b2"][None, :]),
        "ng_rep": rep(p["norm_g"]),
        "nb_rep": rep(p["norm_b"]),
        "mhrep": _bf(np.kron(np.eye(4, dtype=np.float32),
                             np.ones((32, 32), np.float32))),
        "onesb32": _bf(np.ones((128, 32), np.float32)),
        "ones1_64": _bf(np.ones((1, 64), np.float32)),
        "ones1_128": _bf(np.ones((1, 128), np.float32)),
    }
    for l in range(L):
        g1 = float(p["gate1"][l]) * 0.5
        g2 = float(p["gate2"][l])
        obp = (p["vb"][l] @ p["ow"][l] + p["ob"][l]) * g1
        shared.update({
            f"qw{l}": _bf(p["qw"][l].reshape(ND, 128, D)),
            f"kw{l}": _bf(p["kw"][l].reshape(ND, 128, D)),
            f"vw{l}": _bf(p["vw"][l].reshape(ND, 128, D)),
            f"ow{l}": _bf((p["ow"][l] * g1).reshape(ND, 128, D)),
            f"owlo{l}": _bf((p["ow"][l] * g1).reshape(ND, 128, D).astype(np.float32)
                            - _bf((p["ow"][l] * g1).reshape(ND, 128, D)).astype(np.float32)),
            f"f1w{l}": _bf(p["f1w"][l].reshape(ND, 128, FF)),
            f"f2w{l}": _bf((p["f2w"][l] * g2).reshape(NFF, 128, D)),
            f"f2wlo{l}": _bf((p["f2w"][l] * g2).reshape(NFF, 128, D).astype(np.float32)
                             - _bf((p["f2w"][l] * g2).reshape(NFF, 128, D)).astype(np.float32)),
            f"qb2_{l}": np.ascontiguousarray(p["qb"][l].reshape(ND, 128).T),
            f"kb2_{l}": np.ascontiguousarray(p["kb"][l].reshape(ND, 128).T),
            f"obp{l}": _bf(obp[None, :]),
            f"f1b8_{l}": np.ascontiguousarray(p["f1b"][l].reshape(NFF, 128).T),
            f"f2brow{l}": _bf((p["f2b"][l] * g2)[None, :]),
            f"g1rep{l}": rep(p["ln1_g"][l]),
            f"b1rep{l}": rep(p["ln1_b"][l]),
            f"g2rep{l}": rep(p["ln2_g"][l]),
            f"b2rep{l}": rep(p["ln2_b"][l]),
        })

    in_maps = []
    for c in range(NCORES):
        b, half = c // 2, c % 2
        xo = np.zeros(TP, np.float32)
        xo[:TL] = x[b, half * TL:(half + 1) * TL]
        vmo = np.zeros(TP, np.float32)
        vmo[:TL] = vm[b, half * TL:(half + 1) * TL]
        lnmask = np.full(S, -1e9, np.float32)
        vmfull = np.zeros(S, np.float32)
        vmfull[:TL] = vm[b, :TL]
        vmfull[TP:TP + TL] = vm[b, TL:]
        lnmask[vmfull > 0] = 0.0
        poso = np.zeros((TP, D), np.float32)
        poso[:TL] = 0.1 * p["pos"][half * TL:half * TL + TL]
        m = dict(shared)
        m.update({
            "xcol": np.ascontiguousarray(xo.reshape(NT, 128).T),
            "vmrow": _bf(vmo[None, :]),
            "vmsb": _bf(np.ascontiguousarray(vmo.reshape(NT, 128).T)),
            "vmsbf": np.ascontiguousarray(vmo.reshape(NT, 128).T),
            "lnm": np.ascontiguousarray(lnmask.reshape(NS, 128).T),
            "pos_s": _bf(poso),
        })
        in_maps.append(m)
    return in_maps, p, vm


_PER_CORE_INPUTS = ("xcol", "vmrow", "vmsb", "lnm", "pos_s")


def _get_runner(nc):
    """Build a cached jitted SPMD callable.  Shared weights use replicated
    in_specs so they are uploaded once instead of 8x."""
    if "runner" in _CACHE:
        return _CACHE["runner"]
    import jax
    from jax.sharding import Mesh, PartitionSpec as P
    try:
        from jax.experimental.shard_map import shard_map
    except ImportError:
        from jax.shard_map import shard_map  # newer jax
    from concourse import bass2jax

    in_names, out_names, out_avals, zero_outs = [], [], [], []
    partition_name = nc.partition_id_tensor.name if nc.partition_id_tensor else None
    for alloc in nc.m.functions[0].allocations:
        if not isinstance(alloc, mybir.MemoryLocationSet):
            continue
        name = alloc.memorylocations[0].name
        if alloc.kind == "ExternalInput":
            if name != partition_name:
                in_names.append(name)
        elif alloc.kind == "ExternalOutput":
            out_names.append(name)
            shape = tuple(alloc.tensor_shape)
            dtype = mybir.dt.np(alloc.dtype)
            out_avals.append(jax.core.ShapedArray(shape, dtype))
            zero_outs.append(np.zeros(shape, dtype))
    n_params = len(in_names)
    all_in_names = in_names + out_names
    if partition_name is not None:
        all_in_names = all_in_names + [partition_name]

    def _body(*args):
        operands = list(args)
        if partition_name is not None:
            operands.append(bass2jax.partition_id_tensor())
        outs = bass2jax._bass_exec_p.bind(
            *operands,
            out_avals=tuple(out_avals),
            in_names=tuple(all_in_names),
            out_names=tuple(out_names),
            lowering_input_output_aliases=(),
            sim_require_finite=True,
            sim_require_nnan=True,
            nc=nc,
        )
        return tuple(outs)

    devices = jax.devices()[:NCORES]
    mesh = Mesh(np.asarray(devices), ("core",))
    in_specs = tuple(
        P("core") if n in _PER_CORE_INPUTS else P(None) for n in in_names
    ) + (P("core"),) * len(out_names)
    out_specs = (P("core"),) * len(out_names)
    fn = jax.jit(shard_map(_body, mesh=mesh, in_specs=in_specs,
                           out_specs=out_specs, check_rep=False),
                 keep_unused=True)
    runner = dict(fn=fn, in_names=in_names, out_names=out_names,
                  zero_outs=zero_outs, mesh=mesh)
    _CACHE["runner"] = runner
    return runner


def _run_fast(nc, in_maps):
    r = _get_runner(nc)
    args = []
    for n in r["in_names"]:
        if n in _PER_CORE_INPUTS:
            args.append(np.concatenate([m[n] for m in in_maps], axis=0))
        else:
            args.append(in_maps[0][n])
    zeros = [np.zeros((NCORES * z.shape[0],) + z.shape[1:], z.dtype)
             for z in r["zero_outs"]]
    out = r["fn"](*args, *zeros)
    results = []
    for c in range(NCORES):
        d = {}
        for i, n in enumerate(r["out_names"]):
            sh = r["zero_outs"][i].shape
            d[n] = np.asarray(out[i]).reshape((NCORES,) + sh)[c]
        results.append(d)
    return results


def bench(n=10):
    """Time n device executions with inputs already resident on device.
    Requires kernel() to have been called once.  Returns list of seconds."""
    import time
    import jax
    from jax.sharding import NamedSharding, PartitionSpec as P
    r = _CACHE["runner"]
    in_maps = _CACHE["last_in_maps"]
    mesh = r["mesh"]
    dev_args = []
    for name in r["in_names"]:
        if name in _PER_CORE_INPUTS:
            arr = np.concatenate([m[name] for m in in_maps], axis=0)
            sh = NamedSharding(mesh, P("core"))
        else:
            arr = in_maps[0][name]
            sh = NamedSharding(mesh, P(None))
        dev_args.append(jax.device_put(arr, sh))
    for z in r["zero_outs"]:
        arr = np.zeros((NCORES * z.shape[0],) + z.shape[1:], z.dtype)
        dev_args.append(jax.device_put(arr, NamedSharding(mesh, P("core"))))
    out = r["fn"](*dev_args)
    jax.block_until_ready(out)
    times = []
    for _ in range(n):
        t0 = time.perf_counter()
        out = r["fn"](*dev_args)
        jax.block_until_ready(out)
        times.append(time.perf_counter() - t0)
    return times


def kernel(x, validity_mask, params):
    if "nc" not in _CACHE:
        _CACHE["nc"] = _build_program()
    nc = _CACHE["nc"]
    in_maps, p, vm = _prep_inputs(x, validity_mask, params)
    _CACHE["last_in_maps"] = in_maps

    class _Res:
        pass

    res = _Res()
    try:
        res.results = _run_fast(nc, in_maps)
    except Exception:
        res = run_bass_kernel_spmd(nc, in_maps, core_ids=list(range(NCORES)))
    _CACHE["last_results"] = res

    pooled = np.zeros((B, D), np.float32)
    for b in range(B):
        s = res.results[2 * b]["pooled"][0] + res.results[2 * b + 1]["pooled"][0]
        pooled[b] = s / max(float(vm[b].sum()), 1.0)

    binary = _ln_np(pooled, p["bin_ln_g"], p["bin_ln_b"]) @ p["bin_w"] + p["bin_b"]
    anomaly = _ln_np(pooled, p["an_ln_g"], p["an_ln_b"]) @ p["an_w"] + p["an_b"]
    ca = _ln_np(pooled, p["ca_ln_g"], p["ca_ln_b"]) @ p["ca_w"] + p["ca_b"]
    caustic = 1.0 / (1.0 + np.exp(-ca))
    return (binary.astype(np.float32), anomaly.astype(np.float32),
            caustic.astype(np.float32))
